# revision 1
# baseline (speedup 1.0000x reference)
"""Trainium2 Bass kernel for nn_Decoder_83279415869594.

Pipeline (per image): rotate point cloud (ZYZ euler), bilinear-scatter
100k points into a 256x256 grid, 7x7 gaussian conv (separable, SAME
zero-pad), rfft2 -> * ctf -> irfft2.

Implementation: data-parallel over batch B=32 across 8 NeuronCores
(4 images/core).  The scatter is a matmul: for each 128-point tile we
build "hat" operands  Cw[p,y] = v_p*relu(1-|gy_p-y|)  (ScalarE fused
relu-affine with per-partition scale/bias) and  Rm[p,x] = relu(1-|gx_p-x|),
then PE accumulates  img += Cw^T @ Rm  into PSUM (fp16 operands, fp32
accum).  Conv + rfft2 + ctf + irfft2 are dense matmuls against
host-precomputed band/DFT matrices (fp32).
"""

import math
from contextlib import ExitStack

import numpy as np

import concourse.bass as bass
import concourse.tile as tile
from concourse import bacc, mybir
from concourse.bass_utils import run_bass_kernel_spmd

P = 128
X = 256
G = X // 2 + 1  # 129
N_CORES = 8
B_FULL = 32
N_FULL = 100000

f32 = mybir.dt.float32
f16 = mybir.dt.float16
A = mybir.AluOpType
AF = mybir.ActivationFunctionType


# ----------------------------------------------------------------------------
# custom fused DVE ops: +-(min(|in0 - s0|*s1 - s1, 0)) = -+ s1*hat(in0 - s0)
# ----------------------------------------------------------------------------

def _register_hat_ops():
    import concourse.dve_ops as dvo
    from concourse.dve_ops import DveOp
    from concourse.dve_spec import (
        Spec, Src0, C0, C1, Zero, maxx, minn, lower, _has_src1,
    )
    from concourse.dve_uop import DveOpSpec

    def register(name, body, reference, subdim=False):
        for op in dvo.OPS:
            if op.name == name:
                return op
        row = dvo._CUSTOM_DVE_ROW_BASE + len(dvo.OPS)
        assert row < 0x20, "custom-DVE opcode rows exhausted"
        spec = Spec(body=body, reference=reference)
        shas = {}
        for ver in ("v3", "v4"):
            try:
                tmp = DveOpSpec(name=name, opcode=row, uops=lower(spec, ver=ver),
                                rd1_en=_has_src1(spec))
                shas[ver] = tmp.sha(ver)
            except Exception:
                pass
        op = DveOp(name, spec, subdim=subdim, uops_sha=shas)
        dvo.OPS.append(op)
        dvo._SUB_OPCODE_FOR_NAME[name] = row
        dvo.CUSTOM_DVE_SPECS[name] = spec
        return op

    neg_body = minn(maxx(Src0 - C0, C0 - Src0) * C1 - C1, Zero)
    hatneg = register(
        "HATNEG_ANT", neg_body,
        lambda in0, in1, c0, c1, c2: np.minimum(
            np.abs(in0.astype(np.float32) - c0) * c1 - c1, 0))
    hatpos = register(
        "HATPOS_ANT",
        Zero - minn(maxx(Src0 - C0, C0 - Src0) * C1 - C1, Zero),
        lambda in0, in1, c0, c1, c2: -np.minimum(
            np.abs(in0.astype(np.float32) - c0) * c1 - c1, 0))

    # paired R-side op: in0 [P, S, N]; per-page scalar pg = s0 + s*s1
    from concourse.dve_spec import One, PageIdx

    def _ref_r2(sign):
        def r(in0, in1, c0, c1, c2):
            Pp, S, N = in0.shape
            s_idx = np.arange(S).reshape(1, S, 1)
            pg = np.asarray(c0).reshape(-1, 1, 1) + s_idx * np.asarray(c1).reshape(-1, 1, 1)
            return sign * np.minimum(np.abs(in0.astype(np.float32) - pg) - 1.0, 0)
        return r

    pg = PageIdx(C0, C1)
    r2body = minn(maxx(Src0 - pg, pg - Src0) - One, Zero)
    hatr2neg = register("HATR2NEG_ANT", r2body, _ref_r2(1.0), subdim=True)
    hatr2pos = register("HATR2POS_ANT", Zero - r2body, _ref_r2(-1.0), subdim=True)
    return hatneg, hatpos, hatr2neg, hatr2pos


HATNEG, HATPOS, HATR2NEG, HATR2POS = _register_hat_ops()

# scatter schedule: "split" (C custom-DVE, R alternates ACT-pair/custom),
# "alldve" (everything custom-DVE), "rpair" (C custom, R in paired [P,2,X]
# custom ops, ACT takes every 4th-tile pair's R)
SCHED = "split"


# ----------------------------------------------------------------------------
# device program
# ----------------------------------------------------------------------------

def _emit(nc, d, n_img, n_tile, repeat):
    NT = n_tile
    with tile.TileContext(nc) as tc, ExitStack() as ctx:
        const = ctx.enter_context(tc.tile_pool(name="const", bufs=1))
        ppool = ctx.enter_context(tc.tile_pool(name="proj", bufs=2))
        wa = ctx.enter_context(tc.tile_pool(name="wa", bufs=8))
        wb = ctx.enter_context(tc.tile_pool(name="wb", bufs=8))
        fsb = ctx.enter_context(tc.tile_pool(name="fsb", bufs=2))
        psc = ctx.enter_context(tc.tile_pool(name="psc", bufs=2, space="PSUM"))
        pfft = ctx.enter_context(tc.tile_pool(name="pfft", bufs=2, space="PSUM"))

        def load(name, shape, src, dtype=f32):
            t = const.tile(shape, dtype, tag=name)
            nc.sync.dma_start(t[:], src)
            return t

        iota = load("iota", [P, X], d["iota"][:])
        bm = [load(f"bm{k}", [P, X], d["bm"][k * P:(k + 1) * P, :]) for k in range(2)]
        wre = [load(f"wre{k}", [P, X], d["wre"][k * P:(k + 1) * P, :]) for k in range(2)]
        wim = [load(f"wim{k}", [P, X], d["wim"][k * P:(k + 1) * P, :]) for k in range(2)]
        wimneg = [load(f"wimneg{k}", [P, X], d["wimneg"][k * P:(k + 1) * P, :]) for k in range(2)]
        wrre = [load(f"wrre{k}", [P, G], d["wrre"][k * P:(k + 1) * P, :]) for k in range(2)]
        wrim = [load(f"wrim{k}", [P, G], d["wrim"][k * P:(k + 1) * P, :]) for k in range(2)]
        wrimneg = [load(f"wrimneg{k}", [P, G], d["wrimneg"][k * P:(k + 1) * P, :]) for k in range(2)]
        ac = [load("ac0", [P, X], d["ac"][0:P, :]), load("ac1", [1, X], d["ac"][P:G, :])]
        as_ = [load("as0", [P, X], d["as"][0:P, :]), load("as1", [1, X], d["as"][P:G, :])]
        cx = load("cx", [P, NT], d["pts"][0])
        cy = load("cy", [P, NT], d["pts"][1])
        cz = load("cz", [P, NT], d["pts"][2])
        v = load("v", [P, NT], d["pts"][3])
        vneg = load("vneg", [P, NT], d["pts"][4])
        rot = load("rot", [P, 8 * n_img], d["rot"][:])
        ctf_sb = [
            [load(f"ctf{i}_{k}", [P, G], d["ctf"][i, k * P:(k + 1) * P, :]) for k in range(2)]
            for i in range(n_img)
        ]

        def mstep(tag, curs, rhss, out_free, curs2=None, rhss2=None,
                  m_sizes=(P, P), ctf_mul=None):
            """out[m] = sum_k curs[k][:, mslice]^T @ rhss[k]  (+ second term).

            Returns list of SBUF fp32 tiles per m-chunk.  If ctf_mul is
            given, the PSUM result is multiplied elementwise by the given
            SBUF tiles on the way out (one per m-chunk).
            """
            outs = []
            moff = 0
            total = len(curs) * (2 if curs2 is not None else 1)
            for mi, msz in enumerate(m_sizes):
                pm = pfft.tile([msz, out_free], f32, tag=f"pm{mi}")
                nmm = 0
                for k in range(len(curs)):
                    nc.tensor.matmul(pm[:], curs[k][:, moff:moff + msz], rhss[k][:],
                                     start=(nmm == 0), stop=(nmm == total - 1))
                    nmm += 1
                if curs2 is not None:
                    for k in range(len(curs2)):
                        nc.tensor.matmul(pm[:], curs2[k][:, moff:moff + msz], rhss2[k][:],
                                         start=(nmm == 0), stop=(nmm == total - 1))
                        nmm += 1
                sb = fsb.tile([msz, out_free], f32, tag=f"{tag}{mi}")
                if ctf_mul is not None:
                    nc.vector.tensor_tensor(sb[:], pm[:], ctf_mul[mi][:], A.mult)
                else:
                    nc.vector.tensor_copy(sb[:], pm[:])
                outs.append(sb)
                moff += msz
            return outs

        def body():
            for i in range(n_img):
                def rc(j):
                    return rot[:, 8 * i + j:8 * i + j + 1]

                # rot x-rows are NEGATED (for ACT Abs bias), y-rows positive
                gxn = ppool.tile([P, NT], f32, tag="gxn")
                gyp = ppool.tile([P, NT], f32, tag="gyp")
                tq = ppool.tile([P, NT], f32, tag="tq")
                # -gx = cx*(-R00) + cy*(-R01) + cz*(-R02) - (sx + X/2)
                nc.vector.tensor_scalar(tq[:], cx[:], rc(0), None, A.mult)
                nc.vector.scalar_tensor_tensor(gxn[:], cy[:], rc(1), tq[:], A.mult, A.add)
                nc.vector.scalar_tensor_tensor(tq[:], cz[:], rc(2), gxn[:], A.mult, A.add)
                nc.vector.tensor_scalar(gxn[:], tq[:], rc(6), None, A.add)
                gxp = ppool.tile([P, NT], f32, tag="gxp")
                nc.vector.tensor_scalar(gxp[:], gxn[:], -1.0, None, A.mult)
                dgx = None
                if SCHED == "rpair":
                    dgx = ppool.tile([P, NT], f32, tag="dgx")
                    nc.vector.tensor_tensor(dgx[:, 0:NT - 1], gxp[:, 1:NT],
                                            gxp[:, 0:NT - 1], A.subtract)
                # +gy
                nc.vector.tensor_scalar(tq[:], cx[:], rc(3), None, A.mult)
                nc.vector.scalar_tensor_tensor(gyp[:], cy[:], rc(4), tq[:], A.mult, A.add)
                nc.vector.scalar_tensor_tensor(tq[:], cz[:], rc(5), gyp[:], A.mult, A.add)
                nc.vector.tensor_scalar(gyp[:], tq[:], rc(7), None, A.add)

                # ---- scatter: img[y,x] += v * hat(gy-y) * hat(gx-x) ----
                # C-side: one fused custom DVE op -> +-v*hat(iota-gy) (f16).
                # R-side: ACT Abs -> |iota-gx|, then hat via ACT Relu (even
                # tiles, +) or DVE min (odd tiles, -).  Signs per tile cancel
                # in the matmul; PSUM accumulates +v*hat*hat either way.
                ptop = psc.tile([P, X], f32, tag="ptop")
                pbot = psc.tile([P, X], f32, tag="pbot")
                def emit_mm(t, Cw_ap, Rm_ap):
                    nc.tensor.matmul(ptop[:], Cw_ap[:, 0:P], Rm_ap,
                                     start=(t == 0), stop=(t == NT - 1))
                    nc.tensor.matmul(pbot[:], Cw_ap[:, P:X], Rm_ap,
                                     start=(t == 0), stop=(t == NT - 1))

                def r_on_act(t):
                    # R on ACT: |iota-gx| then relu(1-|t|) = +hat
                    aR = wa.tile([P, X], f32, tag="aR")
                    nc.scalar.activation(aR[:], iota[:], AF.Abs,
                                         bias=gxn[:, t:t + 1], scale=1.0)
                    Rm = wb.tile([P, X], f16, tag="Rm")
                    nc.scalar.activation(Rm[:], aR[:], AF.Relu,
                                         bias=1.0, scale=-1.0)
                    return Rm

                def c_custom(t, op):
                    Cw = wb.tile([P, X], f16, tag="Cw")
                    nc.vector._custom_dve(op, out=Cw[:], in0=iota[:],
                                          s0=gyp[:, t:t + 1], s1=v[:, t:t + 1])
                    return Cw

                if SCHED == "split":
                    for t in range(NT):
                        pos = (t % 2 == 0)
                        Cw = c_custom(t, HATPOS if pos else HATNEG)
                        if pos:
                            Rm = r_on_act(t)
                        else:
                            Rm = wb.tile([P, X], f16, tag="Rm")
                            nc.vector._custom_dve(HATNEG, out=Rm[:], in0=iota[:],
                                                  s0=gxp[:, t:t + 1], s1=1.0)
                        emit_mm(t, Cw, Rm[:])
                elif SCHED == "alldve":
                    for t in range(NT):
                        Cw = c_custom(t, HATNEG)
                        Rm = wb.tile([P, X], f16, tag="Rm")
                        nc.vector._custom_dve(HATNEG, out=Rm[:], in0=iota[:],
                                              s0=gxp[:, t:t + 1], s1=1.0)
                        emit_mm(t, Cw, Rm[:])
                elif SCHED == "rpair":
                    assert NT % 2 == 0
                    for t0 in range(0, NT, 2):
                        q = t0 // 2
                        dve_r = (q % 2 == 0)
                        cop = HATNEG if dve_r else HATPOS
                        Cws = [c_custom(t0, cop), c_custom(t0 + 1, cop)]
                        if dve_r:
                            Rm2 = wb.tile([P, 2 * X], f16, tag="Rm2")
                            nc.vector._custom_dve(
                                HATR2NEG,
                                out=Rm2[:].rearrange("p (s n) -> p s n", s=2),
                                in0=iota[:, None, :].broadcast_to([P, 2, X]),
                                s0=gxp[:, t0:t0 + 1], s1=dgx[:, t0:t0 + 1])
                            Rms = [Rm2[:, 0:X], Rm2[:, X:2 * X]]
                        else:
                            Rms = [r_on_act(t0)[:], r_on_act(t0 + 1)[:]]
                        for dt_ in range(2):
                            emit_mm(t0 + dt_, Cws[dt_], Rms[dt_])
                else:
                    raise ValueError(SCHED)

                img = []
                for k, pp in enumerate((ptop, pbot)):
                    sb = fsb.tile([P, X], f32, tag=f"img{k}")
                    nc.vector.tensor_copy(sb[:], pp[:])
                    img.append(sb)

                # ---- conv + rfft2 + ctf + irfft2 as matmul chain ----
                a1 = mstep("a1", img, bm, X)            # [x, y']
                a2 = mstep("a2", a1, bm, X)             # [y', x']
                a3r = mstep("a3r", a2, wre, X)          # [x, f]
                a3i = mstep("a3i", a2, wim, X)
                fpr = mstep("fpr", a3r, wrre, G, curs2=a3i, rhss2=wrimneg,
                            ctf_mul=ctf_sb[i])          # [f, g] * ctf
                fpi = mstep("fpi", a3r, wrim, G, curs2=a3i, rhss2=wrre,
                            ctf_mul=ctf_sb[i])
                a5r = mstep("a5r", fpr, wre, X, curs2=fpi, rhss2=wim,
                            m_sizes=(P, 1))             # [g, y]
                a5i = mstep("a5i", fpi, wre, X, curs2=fpr, rhss2=wimneg,
                            m_sizes=(P, 1))
                outs = mstep("o", a5r, ac, X, curs2=a5i, rhss2=as_)   # [y, x]
                for yc in range(2):
                    nc.sync.dma_start(d["out"][i, yc * P:(yc + 1) * P, :], outs[yc][:])

        if repeat > 1:
            with tc.For_i(0, repeat, 1):
                body()
        else:
            body()


# ----------------------------------------------------------------------------
# host-side constants
# ----------------------------------------------------------------------------

def _euler_rows(ang):
    """Rows 0 and 1 of the ZYZ rotation matrices; ang [B,3] float32."""
    rot, tilt, psi = ang[:, 0].astype(np.float64), ang[:, 1].astype(np.float64), ang[:, 2].astype(np.float64)
    ca, sa = np.cos(rot), np.sin(rot)
    cb, sb = np.cos(tilt), np.sin(tilt)
    cg, sg = np.cos(psi), np.sin(psi)
    cc, cs = cb * ca, cb * sa
    row0 = np.stack([cg * cc - sg * sa, cg * cs + sg * ca, -cg * sb], -1)
    row1 = np.stack([-sg * cc - cg * sa, -sg * cs + cg * ca, sg * sb], -1)
    return np.stack([row0, row1], -2).astype(np.float32)  # [B,2,3]


def _make_consts(gauss_kernel):
    g1n = np.asarray(gauss_kernel, np.float64).sum(axis=0)  # normalized 1D kernel
    K = g1n.shape[0]
    half = K // 2
    Bm = np.zeros((X, X), np.float64)
    for dd in range(-half, half + 1):
        idx = np.arange(max(0, -dd), min(X, X - dd))
        Bm[idx, idx + dd] = g1n[dd + half]
    kk = np.arange(X)
    ang = 2 * np.pi * np.outer(kk, kk) / X
    Wre, Wim = np.cos(ang), -np.sin(ang)
    gg = np.arange(G)
    angr = 2 * np.pi * np.outer(kk, gg) / X
    Wrre, Wrim = np.cos(angr), -np.sin(angr)
    wg = np.where((gg == 0) | (gg == X // 2), 1.0, 2.0)
    angi = 2 * np.pi * np.outer(gg, kk) / X
    Ac = wg[:, None] * np.cos(angi) / (X * X)
    As = -wg[:, None] * np.sin(angi) / (X * X)
    c = {
        "bm": Bm, "wre": Wre, "wim": Wim, "wimneg": -Wim,
        "wrre": Wrre, "wrim": Wrim, "wrimneg": -Wrim, "ac": Ac, "as": As,
    }
    c = {k: np.ascontiguousarray(v, np.float32) for k, v in c.items()}
    c["iota"] = np.ascontiguousarray(
        np.broadcast_to(np.arange(X, dtype=np.float32), (P, X)))
    return c


# ----------------------------------------------------------------------------
# compile cache + public entry point
# ----------------------------------------------------------------------------

_CACHE = {}


def get_program(n_img, n_tile, repeat=1):
    key = (n_img, n_tile, repeat)
    if key in _CACHE:
        return _CACHE[key]
    nc = bacc.Bacc("TRN2", target_bir_lowering=False, debug=False,
                   num_devices=N_CORES)
    NT = n_tile
    d = {
        "pts": nc.dram_tensor("pts", [5, P, NT], f32, kind="ExternalInput").ap(),
        "rot": nc.dram_tensor("rot", [P, 8 * n_img], f32, kind="ExternalInput").ap(),
        "ctf": nc.dram_tensor("ctf", [n_img, X, G], f32, kind="ExternalInput").ap(),
        "iota": nc.dram_tensor("iota", [P, X], f32, kind="ExternalInput").ap(),
        "bm": nc.dram_tensor("bm", [X, X], f32, kind="ExternalInput").ap(),
        "wre": nc.dram_tensor("wre", [X, X], f32, kind="ExternalInput").ap(),
        "wim": nc.dram_tensor("wim", [X, X], f32, kind="ExternalInput").ap(),
        "wimneg": nc.dram_tensor("wimneg", [X, X], f32, kind="ExternalInput").ap(),
        "wrre": nc.dram_tensor("wrre", [X, G], f32, kind="ExternalInput").ap(),
        "wrim": nc.dram_tensor("wrim", [X, G], f32, kind="ExternalInput").ap(),
        "wrimneg": nc.dram_tensor("wrimneg", [X, G], f32, kind="ExternalInput").ap(),
        "ac": nc.dram_tensor("ac", [G, X], f32, kind="ExternalInput").ap(),
        "as": nc.dram_tensor("as", [G, X], f32, kind="ExternalInput").ap(),
        "out": nc.dram_tensor("out", [n_img, X, X], f32, kind="ExternalOutput").ap(),
    }
    _emit(nc, d, n_img, n_tile, repeat)
    nc.compile()
    _CACHE[key] = nc
    return nc


def make_in_maps(alignment, shifts, coords, values, gauss_kernel, ctf,
                 n_img, n_tile, n_cores=N_CORES):
    """Build the per-core input dicts."""
    NT = n_tile
    npts = NT * P
    n_use = min(npts, coords.shape[0])
    cpad = np.zeros((npts, 3), np.float32)
    cpad[:n_use] = np.asarray(coords, np.float32)[:n_use]
    vpad = np.zeros((npts,), np.float32)
    vpad[:n_use] = np.asarray(values, np.float32)[:n_use]
    pts = np.empty((5, P, NT), np.float32)
    for j in range(3):
        pts[j] = cpad[:, j].reshape(P, NT)
    pts[3] = vpad.reshape(P, NT)
    pts[4] = -pts[3]

    R2 = _euler_rows(np.asarray(alignment, np.float32))      # [B,2,3]
    sh = np.asarray(shifts, np.float32)
    consts = _make_consts(gauss_kernel)
    ctf = np.ascontiguousarray(np.asarray(ctf, np.float32))

    in_maps = []
    for c in range(n_cores):
        # x-row negated (device computes -gx for the ACT Abs bias),
        # y-row positive (custom hat op takes +gy)
        rotp = np.zeros((8 * n_img,), np.float32)
        for i in range(n_img):
            b = c * n_img + i
            rotp[8 * i:8 * i + 3] = -R2[b, 0]
            rotp[8 * i + 3:8 * i + 6] = R2[b, 1]
            rotp[8 * i + 6] = -(sh[b, 0] + X / 2.0)
            rotp[8 * i + 7] = sh[b, 1] + X / 2.0
        m = {
            "pts": pts,
            "rot": np.ascontiguousarray(np.broadcast_to(rotp, (P, 8 * n_img))),
            "ctf": ctf[c * n_img:(c + 1) * n_img],
        }
        m.update(consts)
        in_maps.append(m)
    return in_maps


def kernel(alignment, shifts, coords, values, gauss_kernel, ctf):
    n_img = B_FULL // N_CORES                 # 4
    n_tile = math.ceil(N_FULL / P)            # 782
    nc = get_program(n_img, n_tile)
    in_maps = make_in_maps(alignment, shifts, coords, values, gauss_kernel, ctf,
                           n_img, n_tile)
    res = run_bass_kernel_spmd(nc, in_maps, list(range(N_CORES)))
    out = np.empty((B_FULL, X, X), np.float32)
    for c in range(N_CORES):
        out[c * n_img:(c + 1) * n_img] = res.results[c]["out"]
    return out

